# revision 21
# baseline (speedup 1.0000x reference)
"""RBF causal attention (unnormalized, no softmax denominator) on 8 Trainium2 NeuronCores.

Problem: B=2 H=16 N=2048 D=128 fp32.
  P[m,n] = exp(-s*||q_m - k_n||^2) for m >= n else 0;  O = P @ V
         = exp(2s*(q.k) - s*|k|^2) * exp(-s*|q|^2) masked causally.

Sharding: (b*h) = 32 independent slices -> 4 per core across 8 cores, no comms.

Single flat software-pipelined stream over (slice, strip, bn) so the PE/ACT
pipelines never drain at strip or slice boundaries:
  - chunked n-major DMA loads (4-block pieces) so transposes start early
  - PE-transpose Q,K 128x128 blocks (f32r) staged in PSUM, DVE-drained to
    SBUF; transpose groups interleave into the ACT-bound bn stream as PE filler
  - k_sq/q_sq: GPSIMD square + DVE reduce; ksqb=-s*k_sq (ACT bias),
    eq=exp(-s*q_sq) (ACT) scales the output drains; vb = bf16 cast of V (DVE)
  - two m-strips of 1024 per slice; per (strip, bn):
      ST[n, m] = KT_bn.T @ QT strip   (f32r matmul, fp32 PSUM, 512 halves)
      PT = exp(2s*ST - s*k_sq[n])     (ACT, bf16 out, per-partition bias)
      diag block: PT *= upper-tri mask (GPSIMD, bf16)
      per m-block j >= bn: ACC[m, d] += PT_block.T @ vb_bn  (bf16 matmul,
        P^T block as stationary -> output lands directly in [m, d] layout)
      PSUM allows one open accumulation group per 2KB bank: ACC is two
      1-bank tiles (4 m-blocks each); open at the bank's first write,
      close at its last diag, then DVE-drain the quad with eq scale
  - output DMA per strip half
"""

import os
import sys

import numpy as np

_TRN_REPO = "/opt/trn_rl_repo"
if os.path.isdir(_TRN_REPO) and _TRN_REPO not in sys.path:
    sys.path.insert(0, _TRN_REPO)

import concourse.bass as bass  # noqa: E402
import concourse.mybir as mybir  # noqa: E402
import concourse.tile as tile  # noqa: E402
from concourse import bacc  # noqa: E402
from concourse.bass_utils import run_bass_kernel_spmd  # noqa: E402
from concourse.masks import make_identity, make_upper_triangular  # noqa: E402

B, H, N, D = 2, 16, 2048, 128
SM_SCALE = 0.08838834764831845  # 1/sqrt(D)
NCORES = 8
SLICES = (B * H) // NCORES  # per core
NT = N // 128  # 16 row-blocks per slice

F32 = mybir.dt.float32
F32R = mybir.dt.float32r
BF16 = mybir.dt.bfloat16

_nc_cache = None


def _build_nc():
    nc = bacc.Bacc("TRN2", target_bir_lowering=False, debug=False, num_devices=NCORES)

    q_dram = nc.dram_tensor("q", [SLICES, N, D], F32R, kind="ExternalInput").ap()
    k_dram = nc.dram_tensor("k", [SLICES, N, D], F32R, kind="ExternalInput").ap()
    v_dram = nc.dram_tensor("v", [SLICES, N, D], F32, kind="ExternalInput").ap()
    o_dram = nc.dram_tensor("o", [SLICES, N, D], F32, kind="ExternalOutput").ap()

    with tile.TileContext(nc) as tc:
        singles = tc.alloc_tile_pool(name="singles", bufs=1)
        io = tc.alloc_tile_pool(name="io", bufs=2)
        tqk = tc.alloc_tile_pool(name="tqk", bufs=2)
        vbp = tc.alloc_tile_pool(name="vbp", bufs=2)
        sqp = tc.alloc_tile_pool(name="sqp", bufs=4)
        smalls = tc.alloc_tile_pool(name="smalls", bufs=2)
        ptp = tc.alloc_tile_pool(name="ptp", bufs=4)
        outp = tc.alloc_tile_pool(name="outp", bufs=2)
        # 8-bank PSUM budget: stp 3 x 2 banks (QK tiles + transpose stages
        # share the ring), accp 2 x 1 bank
        stp = tc.alloc_tile_pool(name="stp", bufs=3, space="PSUM")
        accp = tc.alloc_tile_pool(name="accp", bufs=2, space="PSUM")

        ident = singles.tile([128, 128], F32)
        make_identity(nc, ident)
        identr = singles.tile([128, 128], F32R)
        nc.vector.tensor_copy(identr, ident)
        # tri[n, m] = 1.0 where m >= n else 0.0 (keep causal, [n, m] layout)
        tri_f = singles.tile([128, 128], F32)
        make_upper_triangular(nc, tri_f, val=1.0, diag=True)
        tri_bf = singles.tile([128, 128], BF16)
        nc.vector.tensor_copy(tri_bf, tri_f)

        def emit_in_dma(s):
            """Chunked input DMAs (4-block pieces, ordered for fast cold start)."""
            kn = io.tile([128, NT, 128], F32R, name=f"kn{s}", tag="kn")
            qn = io.tile([128, NT, 128], F32R, name=f"qn{s}", tag="qn")
            vn = io.tile([128, NT, 128], F32, name=f"vn{s}", tag="vn")

            def piece(dram, t0, t1, dst, eng=nc.sync):
                eng.dma_start(
                    out=dst[:, t0:t1],
                    in_=dram[s][128 * t0 : 128 * t1].rearrange(
                        "(t p) d -> p t d", p=128
                    ),
                )

            if s == 0:
                # cold start: split across both hwdge rings (scalar ring is
                # empty at t=0, so these issues cannot block later ACTs)
                piece(k_dram, 0, 4, kn)
                piece(q_dram, 0, 4, qn)
                piece(q_dram, 4, 8, qn, nc.scalar)
                piece(v_dram, 0, 4, vn, nc.scalar)
                piece(k_dram, 4, 8, kn)
                piece(v_dram, 4, 8, vn, nc.scalar)
                piece(k_dram, 8, 12, kn)
                piece(k_dram, 12, 16, kn)
                piece(q_dram, 8, 12, qn)
                piece(q_dram, 12, 16, qn)
                piece(v_dram, 8, 12, vn)
                piece(v_dram, 12, 16, vn)
                return kn, qn, vn
            piece(k_dram, 0, 4, kn)
            piece(q_dram, 0, 4, qn)
            piece(q_dram, 4, 8, qn)
            piece(k_dram, 4, 8, kn)
            piece(v_dram, 0, 4, vn)
            piece(v_dram, 4, 8, vn)
            piece(k_dram, 8, 12, kn)
            piece(k_dram, 12, 16, kn)
            piece(q_dram, 8, 12, qn)
            piece(q_dram, 12, 16, qn)
            piece(v_dram, 8, 12, vn)
            piece(v_dram, 12, 16, vn)
            return kn, qn, vn

        def transpose_group(s, which, g):
            """PE-transpose blocks 4g..4g+3 of kn/qn into kt/qt[:, 512g:...]
            via a PSUM stage (shared stp ring), drained by DVE."""
            src = sl[s]["io"][0] if which == "k" else sl[s]["io"][1]
            dst = sl[s]["kt"] if which == "k" else sl[s]["qt"]
            stg = stp.tile([128, 512], F32R, name=f"tsg{s}_{which}_{g}", tag="st")
            for j in range(4):
                nc.tensor.transpose(
                    stg[:, 128 * j : 128 * (j + 1)], src[:, 4 * g + j, :], identr
                )
            nc.vector.tensor_copy(dst[:, 512 * g : 512 * (g + 1)], stg)

        sl = {}

        sqt = {}

        def sq_k(s, g):
            """k squares (GPSIMD) for 4-block chunk g."""
            kn = sl[s]["io"][0]
            c0 = 4 * g
            sqk = sqp.tile([128, 4, 128], F32, name=f"sq{s}_k{g}", tag="sq")
            nc.gpsimd.tensor_mul(
                sqk, kn.bitcast(F32)[:, c0 : c0 + 4, :],
                kn.bitcast(F32)[:, c0 : c0 + 4, :],
            )
            sqt[(s, "k", g)] = sqk

        def red_k(s, g):
            """k reduce (DVE, scheduled well after its square) + ksqb bias."""
            c0 = 4 * g
            nc.vector.tensor_reduce(
                sl[s]["ksq"][:, c0 : c0 + 4], sqt.pop((s, "k", g)),
                axis=mybir.AxisListType.X, op=mybir.AluOpType.add,
            )
            nc.gpsimd.tensor_scalar_mul(
                sl[s]["ksqb"][:, c0 : c0 + 4], sl[s]["ksq"][:, c0 : c0 + 4],
                -SM_SCALE,
            )

        def sq_q(s, g):
            """q squares (GPSIMD) for 4-block chunk g."""
            qn = sl[s]["io"][1]
            c0 = 4 * g
            sqq = sqp.tile([128, 4, 128], F32, name=f"sq{s}_q{g}", tag="sq")
            nc.gpsimd.tensor_mul(
                sqq, qn.bitcast(F32)[:, c0 : c0 + 4, :],
                qn.bitcast(F32)[:, c0 : c0 + 4, :],
            )
            sqt[(s, "q", g)] = sqq

        def red_q(s, g):
            """q reduce (DVE) + eq drain scale (ACT)."""
            c0 = 4 * g
            nc.vector.tensor_reduce(
                sl[s]["qsq"][:, c0 : c0 + 4], sqt.pop((s, "q", g)),
                axis=mybir.AxisListType.X, op=mybir.AluOpType.add,
            )
            nc.scalar.activation(
                sl[s]["eq"][:, c0 : c0 + 4], sl[s]["qsq"][:, c0 : c0 + 4],
                mybir.ActivationFunctionType.Exp, scale=-SM_SCALE,
            )

        def prep_v(s, g):
            """bf16 cast of V chunk g (DVE)."""
            nc.vector.tensor_copy(
                sl[s]["vb"][:, 4 * g : 4 * (g + 1), :],
                sl[s]["io"][2][:, 4 * g : 4 * (g + 1), :],
            )

        def alloc_slice(s):
            sl[s] = dict(
                io=in_tiles.pop(s),
                kt=tqk.tile([128, N], F32R, name=f"kt{s}", tag="kt"),
                qt=tqk.tile([128, N], F32R, name=f"qt{s}", tag="qt"),
                vb=vbp.tile([128, NT, 128], BF16, name=f"vb{s}", tag="vb"),
                ksq=smalls.tile([128, NT], F32, name=f"ksq{s}", tag="ksq"),
                qsq=smalls.tile([128, NT], F32, name=f"qsq{s}", tag="qsq"),
                ksqb=smalls.tile([128, NT], F32, name=f"ksqb{s}", tag="ksqb"),
                eq=smalls.tile([128, NT], F32, name=f"eq{s}", tag="eq"),
                o_out=outp.tile([128, NT, 128], F32, name=f"oo{s}", tag="oo"),
            )

        def qk_exp(s, p, bn):
            """ST = KT_bn.T @ QT strip; PT = bf16 exp(2s*ST - s*k_sq); mask."""
            kt, qt = sl[s]["kt"], sl[s]["qt"]
            off = max(0, 128 * bn - 1024 * p)
            stt = stp.tile([128, 1024], F32, name=f"st{s}_{p}_{bn}", tag="st")
            a = off
            while a < 1024:
                b = 512 if a < 512 else 1024
                nc.tensor.matmul(
                    stt[:, a:b],
                    kt[:, 128 * bn : 128 * (bn + 1)],
                    qt[:, 1024 * p + a : 1024 * p + b],
                    start=True,
                    stop=True,
                )
                a = b
            ptt = ptp.tile([128, 1024], BF16, name=f"pt{s}_{p}_{bn}", tag="pt")
            nc.scalar.activation(
                ptt[:, off:], stt[:, off:], mybir.ActivationFunctionType.Exp,
                bias=sl[s]["ksqb"][:, bn : bn + 1], scale=2.0 * SM_SCALE,
            )
            if bn >= 8 * p:
                nc.gpsimd.tensor_mul(
                    ptt[:, off : off + 128], ptt[:, off : off + 128], tri_bf
                )
            return ptt

        # flat (s, p, bn) pair stream
        pairs = [
            (s, p, bn) for s in range(SLICES) for p in (0, 1)
            for bn in range(8 * p + 8)
        ]
        pidx = {t: i for i, t in enumerate(pairs)}

        fillers = {}

        def add_filler(key, fn):
            fillers.setdefault(pidx[key], []).append(fn)

        # Pool-queue poison control: squares (~1us GPSIMD ops) must never sit
        # ahead of a diag mask the PE is about to need. Masks are emitted at
        # pairs (s,0,*) and (s,1,6)..(s,1,13); the mask-free pool windows are
        # (s,1,0)..(s,1,5) and (s,1,14..15), so ALL of slice s+1's prep
        # squares go there.
        for s in range(SLICES):
            if s == 0:
                # slice 0 second-half prep: crowded cold windows, tolerable
                add_filler((0, 0, 0), lambda: sq_q(0, 0))
                add_filler((0, 0, 1), lambda: (sq_q(0, 1), red_q(0, 0)))
                add_filler((0, 0, 3), lambda: red_q(0, 1))
                add_filler((0, 0, 0), lambda: sq_k(0, 2))
                add_filler((0, 0, 1), lambda: sq_k(0, 3))
                add_filler((0, 0, 2), lambda: (transpose_group(0, "k", 2),
                                               red_k(0, 2), prep_v(0, 2)))
                add_filler((0, 0, 3), lambda: (transpose_group(0, "k", 3),
                                               red_k(0, 3), prep_v(0, 3)))
                add_filler((0, 0, 4), lambda: (transpose_group(0, "q", 2),
                                               sq_q(0, 2)))
                add_filler((0, 0, 5), lambda: (transpose_group(0, "q", 3),
                                               sq_q(0, 3)))
                add_filler((0, 0, 6), lambda: red_q(0, 2))
                add_filler((0, 0, 7), lambda: red_q(0, 3))
            else:
                # second-half transposes still run in this slice's strip0
                add_filler((s, 0, 0), lambda s=s: red_k(s, 3))
                add_filler((s, 0, 1), lambda s=s: red_q(s, 2))
                add_filler((s, 0, 2), lambda s=s: (transpose_group(s, "k", 2),
                                                   red_q(s, 3)))
                add_filler((s, 0, 3), lambda s=s: transpose_group(s, "k", 3))
                add_filler((s, 0, 4), lambda s=s: transpose_group(s, "q", 2))
                add_filler((s, 0, 5), lambda s=s: transpose_group(s, "q", 3))
            if s + 1 < SLICES:
                add_filler((s, 0, 0), lambda s=s: in_tiles.update(
                    {s + 1: emit_in_dma(s + 1)}))
                add_filler((s, 1, 0), lambda s=s: (alloc_slice(s + 1),
                                                   sq_k(s + 1, 0)))
                add_filler((s, 1, 1), lambda s=s: sq_k(s + 1, 1))
                add_filler((s, 1, 2), lambda s=s: sq_q(s + 1, 0))
                add_filler((s, 1, 3), lambda s=s: (sq_q(s + 1, 1),
                                                   red_k(s + 1, 0)))
                add_filler((s, 1, 4), lambda s=s: (transpose_group(s + 1, "q", 0),
                                                   red_k(s + 1, 1),
                                                   prep_v(s + 1, 0)))
                add_filler((s, 1, 5), lambda s=s: (sq_k(s + 1, 2),
                                                   red_q(s + 1, 0)))
                add_filler((s, 1, 6), lambda s=s: (transpose_group(s + 1, "k", 0),
                                                   red_q(s + 1, 1),
                                                   prep_v(s + 1, 1)))
                add_filler((s, 1, 8), lambda s=s: (transpose_group(s + 1, "q", 1),
                                                   red_k(s + 1, 2)))
                add_filler((s, 1, 10), lambda s=s: (transpose_group(s + 1, "k", 1),
                                                    prep_v(s + 1, 2)))
                add_filler((s, 1, 14), lambda s=s: (sq_q(s + 1, 2),
                                                    sq_k(s + 1, 3)))
                add_filler((s, 1, 15), lambda s=s: (sq_q(s + 1, 3),
                                                    prep_v(s + 1, 3)))

        # cold start: slice 0 DMA, first-half transposes + prep
        in_tiles = {0: emit_in_dma(0)}
        alloc_slice(0)
        transpose_group(0, "q", 0)
        transpose_group(0, "k", 0)
        transpose_group(0, "q", 1)
        transpose_group(0, "k", 1)
        sq_k(0, 0)
        sq_k(0, 1)
        prep_v(0, 0)
        red_k(0, 0)
        prep_v(0, 1)
        red_k(0, 1)

        acc = {}
        pts = {0: qk_exp(*pairs[0]), 1: qk_exp(*pairs[1])}
        for i, (s, p, bn) in enumerate(pairs):
            if i + 2 < len(pairs):
                pts[i + 2] = qk_exp(*pairs[i + 2])
            ptt = pts.pop(i)
            if bn == 0:
                acc[0] = accp.tile([128, 4, 128], F32, name=f"acA{s}_{p}", tag="acc")
                acc[1] = accp.tile([128, 4, 128], F32, name=f"acB{s}_{p}", tag="acc")
            j0 = max(0, bn - 8 * p)
            js = list(range(j0, 8))
            if bn >= max(8 * p, 1) and len(js) > 1:
                # diag block last so its mask is off the PE critical path
                js = js[1:] + js[:1]
            for j in js:
                bm = 8 * p + j
                bank_last = (j % 4 == 3) and bn == bm
                nc.tensor.matmul(
                    acc[j // 4][:, j % 4, :],
                    ptt[:, 128 * j : 128 * (j + 1)],
                    sl[s]["vb"][:, bn, :],
                    start=(bn == 0 and j % 4 == 0),
                    stop=bank_last,
                )
                if bank_last:
                    for jj in range(j - 3, j + 1):
                        nc.vector.tensor_scalar_mul(
                            sl[s]["o_out"][:, 8 * p + jj, :],
                            acc[j // 4][:, jj % 4, :],
                            sl[s]["eq"][:, 8 * p + jj : 8 * p + jj + 1],
                        )
            for fn in fillers.get(i, ()):
                fn()
            if bn == 8 * p + 7:  # strip end -> output DMA for this half
                nc.sync.dma_start(
                    out=o_dram[s][1024 * p : 1024 * (p + 1)].rearrange(
                        "(t p2) d -> p2 t d", p2=128
                    ),
                    in_=sl[s]["o_out"][:, 8 * p : 8 * (p + 1)],
                )

        for pool in (accp, stp, outp, ptp, smalls, sqp, vbp, tqk, io, singles):
            pool.release()

    nc.compile()
    return nc


def _get_nc():
    global _nc_cache
    if _nc_cache is None:
        _nc_cache = _build_nc()
    return _nc_cache


def run(q, k, v, trace=False):
    q = np.ascontiguousarray(np.asarray(q, dtype=np.float32))
    k = np.ascontiguousarray(np.asarray(k, dtype=np.float32))
    v = np.ascontiguousarray(np.asarray(v, dtype=np.float32))
    qf = q.reshape(B * H, N, D)
    kf = k.reshape(B * H, N, D)
    vf = v.reshape(B * H, N, D)
    nc = _get_nc()
    in_maps = [
        {
            "q": np.ascontiguousarray(qf[SLICES * i : SLICES * (i + 1)]),
            "k": np.ascontiguousarray(kf[SLICES * i : SLICES * (i + 1)]),
            "v": np.ascontiguousarray(vf[SLICES * i : SLICES * (i + 1)]),
        }
        for i in range(NCORES)
    ]
    res = run_bass_kernel_spmd(nc, in_maps, core_ids=list(range(NCORES)), trace=trace)
    out = np.concatenate([res.results[i]["o"] for i in range(NCORES)], axis=0)
    return out.reshape(B, H, N, D).astype(np.float32), res


def kernel(q, k, v):
    return run(q, k, v)[0]
